# revision 18
# baseline (speedup 1.0000x reference)
"""E8P codebook dequant kernel for 8x TRN2 NeuronCores (Bass/Tile).

Row-parallel sharding: core c handles rows [512c, 512c+512) of weight_q and
produces the matching [512, 11008] f32 slice of the output. grid and scale
are replicated to every core. No cross-core communication.

On-device algorithm (per core):
  - One SBUF table tensor T [128, 32768] f32 holds the scale-folded codebook,
    split across partition halves of each 16-partition GPSIMD group:
      partitions p with p%16 == j < 8:  T[p][s] = scale*grid[s-1][j]
                                        (s in 1..32767 -> entries 0..32766)
      partitions p with p%16 == 8+j:    T[p][s] = scale*grid[32766+s][j]
                                        (s in 1..32767 -> entries 32767..65533)
      slot 0 is 0.0 everywhere (sentinel; ap_gather clamps negative stream
      values to slot 0).
  - T3 [128, 4] covers the two remaining entries: slots 1,2 hold
    scale*grid[65534/65535][j] on low partitions, 0 elsewhere.
  - Index streams (int16, computed in u16 with wraparound then bitcast):
      s1 = idx + 1               valid for idx in [0, 32766]
      s2 = idx - 32766           valid for idx in [32767, 65533]
      s3 = max(idx, 65533) - 65533   -> 0 / 1 / 2
  - Three ap_gather calls per chunk over the shared tables; merge
      out[16g+j] = (X1 + X3)[16g+j] + X2[16g+8+j]
    where exactly one term is nonzero per element, so f32 adds are exact.
"""

import numpy as np

import concourse.bass as bass
import concourse.bacc as bacc
import concourse.tile as tile
import concourse.mybir as mybir
from concourse.bass_utils import run_bass_kernel_spmd

OUT_F = 4096
IN_F = 11008
CODESZ = 8
CB = 65536
N_CORES = 8

ROWS = OUT_F // N_CORES          # 512 rows per core
QCOLS = IN_F // CODESZ           # 1376 codes per row
N_IDX = ROWS * QCOLS             # 704512 indices per core
PER_PART = N_IDX // 128          # 5504 indices per partition (= 4 rows)

F_CHUNK = 344                    # 1376 = 4 * 344: chunks never cross a row
S_CHUNK = F_CHUNK * 16           # 5504 stream elements per group per call
N_CHUNKS = PER_PART // F_CHUNK   # 16
CHUNKS_PER_ROW = QCOLS // F_CHUNK  # 4

_CACHE: dict = {}
REPEAT = 1  # device-work multiplier (timing experiments only)


def _build():
    if "nc" in _CACHE:
        return _CACHE["nc"]
    dt = mybir.dt
    nc = bacc.Bacc("TRN2", target_bir_lowering=False, debug=False,
                   enable_asserts=False, num_devices=N_CORES,
                   dynamic_dma_scratch_size=2048)
    wq_d = nc.dram_tensor("wq", [ROWS, QCOLS], dt.int32, kind="ExternalInput")
    # grid arrives host-transposed [8, 65536] so table loads are contiguous
    grid_d = nc.dram_tensor("gridT", [CODESZ, CB], dt.float32, kind="ExternalInput")
    scale_d = nc.dram_tensor("scale", [1], dt.float32, kind="ExternalInput")
    out_d = nc.dram_tensor("out", [N_CHUNKS * 8 * 8 * F_CHUNK * 16],
                       dt.float32, kind="ExternalOutput")

    with tile.TileContext(nc) as tc:
        with tc.tile_pool(name="tab", bufs=1) as tabp, \
             tc.tile_pool(name="small", bufs=1) as smallp, \
             tc.tile_pool(name="idx", bufs=1) as idxp, \
             tc.tile_pool(name="st", bufs=1) as stp, \
             tc.tile_pool(name="x1", bufs=1) as x1p, \
             tc.tile_pool(name="x3", bufs=1) as x3p, \
             tc.tile_pool(name="xunused", bufs=1) as x2sp:

            # ---- scale broadcast to all 128 partitions ----
            scale_t = smallp.tile([128, 1], dt.float32)
            nc.sync.dma_start(scale_t[:], bass.AP(scale_d, 0, [[0, 128], [1, 1]]))

            # ---- codebook table T ----
            T = tabp.tile([128, 32768], dt.float32)
            for j in range(8):
                # low half: entries 0..32766 -> slots 1..32767
                nc.sync.dma_start(
                    T[:][j::16, 1:32768],
                    bass.AP(grid_d, j * CB, [[0, 8], [1, 32767]]),
                )
                # high half: entries 32767..65533 -> slots 1..32767
                nc.sync.dma_start(
                    T[:][(8 + j)::16, 1:32768],
                    bass.AP(grid_d, j * CB + 32767, [[0, 8], [1, 32767]]),
                )
            nc.vector.memset(T[:][:, 0:1], 0.0)
            # fold scale into the table (f32, same rounding as reference)
            nc.vector.tensor_scalar(T[:], T[:], scale_t[:], None,
                                    mybir.AluOpType.mult)

            # ---- tiny table T3 for entries 65534, 65535 ----
            T3 = smallp.tile([128, 4], dt.float32)
            nc.vector.memset(T3[:], 0.0)
            for j in range(8):
                nc.sync.dma_start(
                    T3[:][j::16, 1:3],
                    bass.AP(grid_d, j * CB + 65534, [[0, 8], [1, 2]]),
                )
            nc.vector.tensor_scalar(T3[:], T3[:], scale_t[:], None,
                                    mybir.AluOpType.mult)

            add = mybir.AluOpType.add
            sub = mybir.AluOpType.subtract
            mx = mybir.AluOpType.max

            for u in [u for _ in range(REPEAT) for u in range(N_CHUNKS // 2)]:
                ta, tb = 2 * u, 2 * u + 1
                # load both chunks' codes up front for the paired T3 stream
                wq_a = stp.tile([128, F_CHUNK], dt.int32, tag="wqa")
                wq_b = stp.tile([128, F_CHUNK], dt.int32, tag="wqb")
                nc.sync.dma_start(
                    wq_a[:],
                    bass.AP(wq_d, ta * F_CHUNK, [[PER_PART, 128], [1, F_CHUNK]]))
                nc.sync.dma_start(
                    wq_b[:],
                    bass.AP(wq_d, tb * F_CHUNK, [[PER_PART, 128], [1, F_CHUNK]]))
                ida = wq_a[:].bitcast(dt.uint16)[:, 0::2]
                idb = wq_b[:].bitcast(dt.uint16)[:, 0::2]

                # one T3 gather covers both chunks (amortizes call overhead)
                s3p = stp.tile([128, 2 * F_CHUNK], dt.int16, tag="s3p")
                nc.vector.tensor_scalar(
                    s3p[:].bitcast(dt.uint16)[:, 0:F_CHUNK], ida,
                    65533, 65533, mx, sub)
                nc.vector.tensor_scalar(
                    s3p[:].bitcast(dt.uint16)[:, F_CHUNK:], idb,
                    65533, 65533, mx, sub)
                X3p = x3p.tile([128, 2 * S_CHUNK], dt.float32)
                nc.gpsimd.ap_gather(X3p[:], T3[:], s3p[:], channels=128,
                                    num_elems=4, d=1, num_idxs=2 * S_CHUNK)

                for (t, idc, x3off) in ((ta, ida, 0), (tb, idb, S_CHUNK)):
                    s12 = stp.tile([128, 2 * F_CHUNK], dt.int16, tag="s12")
                    nc.vector.tensor_scalar(
                        s12[:].bitcast(dt.uint16)[:, 0:F_CHUNK], idc, 1, None, add)
                    nc.vector.tensor_scalar(
                        s12[:].bitcast(dt.uint16)[:, F_CHUNK:], idc, 32766, None, sub)

                    X12 = x1p.tile([128, 2 * S_CHUNK], dt.float32)
                    nc.gpsimd.ap_gather(X12[:], T[:], s12[:], channels=128,
                                        num_elems=32768, d=1, num_idxs=2 * S_CHUNK)
                    X1 = X12[:][:, 0:S_CHUNK]
                    X2 = X12[:][:, S_CHUNK:2 * S_CHUNK]
                    X3c = X3p[:][:, x3off:x3off + S_CHUNK]

                    # in-place partition shift of the high half, then merge
                    shuf = [(8 + i) if (i % 16) < 8 else i for i in range(32)]
                    nc.vector.stream_shuffle(X2, X2, shuf)
                    nc.vector.tensor_add(X3c, X3c, X1)
                    nc.vector.tensor_add(X1, X3c, X2)

                    # ---- planar write back (same layout as before) ----
                    for j in range(8):
                        src_ap = X12[:][j::16, 0:S_CHUNK].rearrange(
                            "p (f pp) -> p f pp", pp=16)
                        blk = 8 * F_CHUNK * 16
                        dst = bass.AP(
                            out_d, (t * 8 + j) * blk,
                            [[F_CHUNK * 16, 8], [16, F_CHUNK], [1, 16]],
                        )
                        nc.sync.dma_start(dst, src_ap)

    nc.compile()
    _CACHE["nc"] = nc
    return nc


def kernel(weight_q: np.ndarray, grid: np.ndarray, scale: np.ndarray) -> np.ndarray:
    weight_q = np.ascontiguousarray(np.asarray(weight_q, dtype=np.int32))
    grid = np.ascontiguousarray(np.asarray(grid, dtype=np.float32))
    scale = np.ascontiguousarray(np.asarray(scale, dtype=np.float32))
    nc = _build()
    grid_t = np.ascontiguousarray(grid.T)   # layout marshalling for replication
    in_maps = []
    for c in range(N_CORES):
        in_maps.append({
            "wq": weight_q[c * ROWS:(c + 1) * ROWS],
            "gridT": grid_t,
            "scale": scale,
        })
    res = run_bass_kernel_spmd(nc, in_maps, core_ids=list(range(N_CORES)))
    shards = []
    for c in range(N_CORES):
        planar = res.results[c]["out"].reshape(N_CHUNKS, 8, 8, F_CHUNK, 16)
        # element (t, j, g, f, pp) -> row 64g + 4pp + t//8,
        #                            col ((t%8)*F_CHUNK + f)*8 + j
        p6 = planar.reshape(4, CHUNKS_PER_ROW, 8, 8, F_CHUNK, 16)  # tt, tq, j, g, f, pp
        # -> [g, pp, tt, tq, f, j]
        x = np.transpose(p6, (3, 5, 0, 1, 4, 2))
        shards.append(x.reshape(ROWS, IN_F))
    return np.concatenate(shards, axis=0)


if __name__ == "__main__":
    rng = np.random.default_rng(0)
    wq = rng.integers(0, CB, size=(OUT_F, QCOLS), dtype=np.int32)
    g = rng.standard_normal((CB, CODESZ)).astype(np.float32)
    s = rng.random(1).astype(np.float32)
    got = kernel(wq, g, s)
    exp = (g[wq].reshape(OUT_F, IN_F) * s).astype(np.float32)
    err = np.abs(got - exp)
    denom = np.maximum(np.abs(exp), 1e-6)
    print("max abs err:", err.max())
    print("max rel err:", (err / denom).max())
    print("exact match:", np.array_equal(got, exp))
